# revision 8
# baseline (speedup 1.0000x reference)
"""Trainium2 Bass kernel for the GPT2Shared multimodal ensemble MLP.

Pipeline (per token): three modality adapters (Linear+GELU) -> shared
projection -> concat -> 32-expert ensemble MLP (2304->768->32->5, relu) ->
mean over experts -> mean over time.

Sharding: pure data-parallel over the batch dim. Each of the 8 cores gets
4 batches (1024 tokens) and runs the whole pipeline for its tokens; the
final reduction over experts+time happens on-device, so each core emits a
[5, 4] partial and the host only rescales/concats.

All on-device tensors live in [feature, token] layout so every matmul uses
the natural weight layout as the stationary operand and no transposes are
needed anywhere.

Precision: the adapters, shared projection, and the dominant ensemble
first layer (2304->768 per expert, ~79% of all FLOPs) run as fp8-e4m3
DoubleRow matmuls (2 fp8 weights per PE cell -> 256-deep contraction per
pass, 2x bf16 throughput). Weights are pre-scaled by 2^7 and the chunk by
2^4 on the way into fp8 so values sit in e4m3's normal range; the scales
are divided back out (exact powers of two) inside the consuming
activation instruction. The narrow ensemble layers 2/3 stay bf16 - fp8
there triples the final error for <4% of the compute. PSUM accumulation
is fp32 throughout. Measured end-to-end rel err ~6.6e-3 (gate 2e-2).
"""

import os
import sys

for _p in ("/opt/trn_rl_repo", "/root/.axon_site/_ro/trn_rl_repo"):
    if os.path.isdir(_p) and _p not in sys.path:
        sys.path.append(_p)

import ml_dtypes
import numpy as np

import concourse.bass as bass
import concourse.tile as tile
from concourse import bacc, mybir
from concourse.bass_utils import run_bass_kernel_spmd

BF16 = mybir.dt.bfloat16
F8 = mybir.dt.float8e4
F32 = mybir.dt.float32
NPBF = ml_dtypes.bfloat16
NPF8 = ml_dtypes.float8_e4m3

N_CORES = 8
B, T = 32, 256
TOK = B * T // N_CORES          # 1024 tokens per core
BPC = B // N_CORES              # 4 batches per core
NT, NSZ = 2, 512                # token tiles per core
GKT = 18                        # 2304 gelu/chunk features = 18 x 128 rows
GK2 = 9                         # ... = 9 x 256 DoubleRow k-tiles
PKT = 6                         # 768 proj features = 6 x 128 rows
E, HID, TGT = 32, 32, 5
SW = 128.0                      # fp8 weight pre-scale (2^7)
SC = 16.0                       # fp8 chunk pre-scale (2^4)
# (name, in_dim, in 256-k-tiles, chunk row-tile offset) in reference
# concat order: chunk = [video, text, audio]
MODS = (("v", 768, 3, 0), ("t", 768, 3, 6), ("a", 1024, 4, 12))

_NC = None
LAST_RESULT = None


def _build():
    nc = bacc.Bacc("TRN2", target_bir_lowering=False, debug=False,
                   num_devices=N_CORES)
    DR = mybir.MatmulPerfMode.DoubleRow

    dr = {}
    for mn, kin, k2t, _ in MODS:
        dr[f"x{mn}"] = nc.dram_tensor(f"x{mn}", [128, k2t, 2, TOK], F8,
                                      kind="ExternalInput")
        dr[f"W{mn}"] = nc.dram_tensor(f"W{mn}", [128, k2t, 2, 2304], F8,
                                      kind="ExternalInput")
        dr[f"b{mn}"] = nc.dram_tensor(f"b{mn}", [128, GKT], F32,
                                      kind="ExternalInput")
    dr["Wp"] = nc.dram_tensor("Wp", [128, GK2, 2, 768], F8, kind="ExternalInput")
    dr["bp"] = nc.dram_tensor("bp", [128, PKT], F32, kind="ExternalInput")
    dr["We1"] = nc.dram_tensor("We1", [E, 128, GK2, 2, 768], F8,
                               kind="ExternalInput")
    dr["be1"] = nc.dram_tensor("be1", [128, E, PKT], F32, kind="ExternalInput")
    # host-rearranged: [p, kt, e, h] <- We2[e, kt*128+p, h]
    dr["We2"] = nc.dram_tensor("We2", [128, PKT, E, HID], BF16, kind="ExternalInput")
    # 4-expert-packed layer-2 bias: be2_pack[q*32+h, g] = be2[4g+q, h]
    dr["be2"] = nc.dram_tensor("be2", [128, 8], F32, kind="ExternalInput")
    # host-stacked: [p, kt, t] <- We3[(kt*128+p)//32, (kt*128+p)%32, t]
    dr["We3"] = nc.dram_tensor("We3", [128, 8, TGT], BF16, kind="ExternalInput")
    out_d = nc.dram_tensor("out", [TGT, BPC], F32, kind="ExternalOutput")

    gelu = mybir.ActivationFunctionType.Gelu_apprx_tanh
    relu = mybir.ActivationFunctionType.Relu
    ident = mybir.ActivationFunctionType.Identity

    with tile.TileContext(nc) as tc:
        with (
            tc.tile_pool(name="const", bufs=1) as constp,
            tc.tile_pool(name="persist", bufs=1) as perp,
            tc.tile_pool(name="adw", bufs=1) as adw,
            tc.tile_pool(name="adwm", bufs=1) as adwm,
            tc.tile_pool(name="adf", bufs=2) as adf,
            tc.tile_pool(name="we1p", bufs=2) as we1p,
            tc.tile_pool(name="h1p", bufs=5) as h1p,
            tc.tile_pool(name="psA", bufs=5, space=bass.MemorySpace.PSUM) as psA,
            tc.tile_pool(name="psB", bufs=2, space=bass.MemorySpace.PSUM) as psB,
            tc.tile_pool(name="psC", bufs=1, space=bass.MemorySpace.PSUM) as psC,
        ):
            # chunk (= concat of the three projected modalities), fp8,
            # scaled by SC, in DoubleRow k-pair layout [p, k2, i, tok]
            chunk_sb = perp.tile([128, GK2, 2, TOK], F8, tag="chunk")

            # ---------------- adapters + shared projection ----------------
            # DMA issue order puts the first modality's weights/features
            # first so the PE can start ~4us into the kernel; constants and
            # Wp follow (not needed until the first gelu psum drains).
            first = True
            for mn, kin, k2t, coff in MODS:
                wm_sb = adwm.tile([128, 4, 2, 2304], F8, tag="wmod")
                f_sb = adf.tile([128, 4, 2, TOK], F8, tag="feat")
                # split per k2-tile: the first matmul only needs slice 0
                for kt in range(k2t):
                    nc.sync.dma_start(wm_sb[:, kt:kt + 1], dr[f"W{mn}"][:, kt:kt + 1])
                    nc.sync.dma_start(f_sb[:, kt:kt + 1], dr[f"x{mn}"][:, kt:kt + 1])
                bm_sb = constp.tile([128, GKT], F32, tag=f"b{mn}")
                nc.sync.dma_start(bm_sb[:], dr[f"b{mn}"][:])
                if first:
                    first = False
                    wp_sb = adw.tile([128, GK2, 2, 768], F8, tag="wp")
                    nc.sync.dma_start(wp_sb[:], dr["Wp"][:])
                    bp_sb = constp.tile([128, PKT], F32, tag="bp")
                    nc.sync.dma_start(bp_sb[:], dr["bp"][:])
                    be1_sb = constp.tile([128, E, PKT], F32, tag="be1")
                    nc.sync.dma_start(be1_sb[:], dr["be1"][:])
                    we2_sb = constp.tile([128, PKT, E, HID], BF16, tag="we2")
                    nc.sync.dma_start(we2_sb[:], dr["We2"][:])
                    be2_sb = constp.tile([128, 8], F32, tag="be2")
                    nc.sync.dma_start(be2_sb[:], dr["be2"][:])
                    we3_sb = constp.tile([128, 8, TGT], BF16, tag="we3")
                    nc.sync.dma_start(we3_sb[:], dr["We3"][:])
                g_sb = adw.tile([128, GK2, 2, TOK], F8, tag="g")
                # g = gelu(x @ Wm + bm), fp8 unscaled, [feature, token]
                for n in range(NT):
                    for gf in range(GKT):
                        ps = psA.tile([128, NSZ], F32, tag="ps")
                        for kt in range(k2t):
                            nc.tensor.matmul(
                                ps[:],
                                wm_sb[:, kt, :, gf * 128:(gf + 1) * 128],
                                f_sb[:, kt, :, n * NSZ:(n + 1) * NSZ],
                                start=(kt == 0), stop=(kt == k2t - 1),
                                perf_mode=DR)
                        nc.scalar.activation(
                            g_sb[:, gf // 2, gf % 2, n * NSZ:(n + 1) * NSZ],
                            ps[:], gelu, bias=bm_sb[:, gf:gf + 1],
                            scale=1.0 / SW)
                # chunk row-tiles [coff:coff+6] = SC * (g @ Wp + bp)
                for n in range(NT):
                    for pf in range(PKT):
                        ps = psA.tile([128, NSZ], F32, tag="ps")
                        for kt in range(GK2):
                            nc.tensor.matmul(
                                ps[:],
                                wp_sb[:, kt, :, pf * 128:(pf + 1) * 128],
                                g_sb[:, kt, :, n * NSZ:(n + 1) * NSZ],
                                start=(kt == 0), stop=(kt == GK2 - 1),
                                perf_mode=DR)
                        r = coff + pf
                        # identity w/ scale+bias on the (otherwise idle) DVE
                        # so the ACT engine keeps pace with the gelu stream
                        nc.vector.tensor_scalar(
                            chunk_sb[:, r // 2, r % 2, n * NSZ:(n + 1) * NSZ],
                            ps[:], SC / SW, bp_sb[:, pf:pf + 1],
                            mybir.AluOpType.mult, mybir.AluOpType.add)

            # ---------------- ensemble ----------------
            h2_sb = perp.tile([128, 8, TOK], BF16, tag="h2")
            if True:
                h1_grp = []
                for e in range(E):
                    w1_sb = we1p.tile([128, GK2, 2, 768], F8, tag="w1")
                    nc.sync.dma_start(w1_sb[:], dr["We1"][e])
                    h1_sb = h1p.tile([128, PKT, TOK], BF16, tag="h1")
                    for n in range(NT):
                        for pf in range(PKT):
                            ps = psA.tile([128, NSZ], F32, tag="ps")
                            for kt in range(GK2):
                                nc.tensor.matmul(
                                    ps[:],
                                    w1_sb[:, kt, :, pf * 128:(pf + 1) * 128],
                                    chunk_sb[:, kt, :, n * NSZ:(n + 1) * NSZ],
                                    start=(kt == 0), stop=(kt == GK2 - 1),
                                    perf_mode=DR)
                            nc.scalar.activation(
                                h1_sb[:, pf, n * NSZ:(n + 1) * NSZ], ps[:],
                                relu, bias=be1_sb[:, e, pf:pf + 1],
                                scale=1.0 / (SC * SW))
                    h1_grp.append(h1_sb)
                    if e % 4 != 3:
                        continue
                    # layer 2 for experts 4g..4g+3 packed into one PE pass:
                    # each expert's [128,32] We2 sits in its own 32-column
                    # group of the array (tile_position from the psum
                    # quarter), so the 4 matmuls per k-tile run concurrently
                    g = e // 4
                    for n in range(NT):
                        ps2 = psB.tile([128, NSZ], F32, tag="ps2")
                        for kt in range(PKT):
                            for q in range(4):
                                nc.tensor.matmul(
                                    ps2[q * 32:(q + 1) * 32, :],
                                    we2_sb[:, kt, 4 * g + q, :],
                                    h1_grp[q][:, kt, n * NSZ:(n + 1) * NSZ],
                                    start=(kt == 0), stop=(kt == PKT - 1),
                                    tile_position=(0, q * 32))
                        nc.scalar.activation(
                            h2_sb[:, g, n * NSZ:(n + 1) * NSZ],
                            ps2[:], relu, bias=be2_sb[:, g:g + 1])
                    h1_grp = []

                # ensemble head: accumulate all 32 experts' 5-dim outputs and
                # reduce over time within each batch
                s_sb = constp.tile([TGT, BPC], F32, tag="s")
                for n in range(NT):
                    ps3 = psC.tile([TGT, NSZ], F32, tag="ps3")
                    for kt in range(8):
                        nc.tensor.matmul(
                            ps3[:],
                            we3_sb[:, kt, :],
                            h2_sb[:, kt, n * NSZ:(n + 1) * NSZ],
                            start=(kt == 0), stop=(kt == 7))
                    nc.vector.reduce_sum(
                        s_sb[:, 2 * n:2 * n + 2],
                        ps3[:].rearrange("p (g t) -> p g t", t=T),
                        axis=mybir.AxisListType.X)
                nc.sync.dma_start(out_d[:], s_sb[:])

    nc.compile()
    return nc


def _pair128(a):
    """[K, M] -> [128, K//256, 2, M]: DoubleRow k-pair layout, partition
    dim first."""
    K, M = a.shape
    return np.ascontiguousarray(
        a.reshape(K // 256, 2, 128, M).transpose(2, 0, 1, 3))


def _prep(inputs):
    """Host-side: cast/scale to fp8 or bf16, transpose feats to
    [feature, token], build per-core input maps."""
    f32 = np.float32

    def bf(x):
        return np.asarray(x, f32).astype(NPBF)

    def f8(x, s=1.0):
        return (np.asarray(x, f32) * f32(s)).astype(NPF8)

    feats = {
        "v": np.asarray(inputs["video_feat"], f32).reshape(B * T, 768),
        "t": np.asarray(inputs["text_feat"], f32).reshape(B * T, 768),
        "a": np.asarray(inputs["audio_feat"], f32).reshape(B * T, 1024),
    }
    # [128, k2t, 2, N] fp8, unscaled (randn fits e4m3 range directly)
    featsT = {k: _pair128(f8(v.T)) for k, v in feats.items()}

    wkeys = {"v": "Wv", "t": "Wt", "a": "Wa"}
    bkeys = {"v": "bv", "t": "bt", "a": "ba"}
    shared = {}
    for mn, kin, _, _ in MODS:
        shared[f"W{mn}"] = _pair128(f8(inputs[wkeys[mn]], SW))
        shared[f"b{mn}"] = np.ascontiguousarray(
            np.asarray(inputs[bkeys[mn]], f32).reshape(GKT, 128).T)
    shared["Wp"] = _pair128(f8(inputs["Wp"], SW))
    shared["bp"] = np.ascontiguousarray(
        np.asarray(inputs["bp"], f32).reshape(PKT, 128).T * f32(SC))
    w1 = f8(inputs["We1"], SW)  # [E, 2304, 768]
    shared["We1"] = np.ascontiguousarray(
        w1.reshape(E, GK2, 2, 128, 768).transpose(0, 3, 1, 2, 4))
    shared["be1"] = np.ascontiguousarray(
        np.asarray(inputs["be1"], f32).reshape(E, PKT, 128).transpose(2, 0, 1))
    shared["We2"] = np.ascontiguousarray(
        bf(inputs["We2"]).reshape(E, PKT, 128, HID).transpose(2, 1, 0, 3))
    shared["be2"] = np.ascontiguousarray(
        np.asarray(inputs["be2"], f32).reshape(8, 4, HID)
        .transpose(1, 2, 0).reshape(128, 8))
    shared["We3"] = np.ascontiguousarray(
        bf(inputs["We3"]).reshape(8, 128, TGT).transpose(1, 0, 2))

    in_maps = []
    for c in range(N_CORES):
        m = dict(shared)
        sl = slice(c * TOK, (c + 1) * TOK)
        for mn, _, _, _ in MODS:
            m[f"x{mn}"] = np.ascontiguousarray(featsT[mn][:, :, :, sl])
        in_maps.append(m)
    be3_sum = np.asarray(inputs["be3"], f32).sum(axis=0)
    return in_maps, be3_sum


def kernel(**inputs):
    global _NC, LAST_RESULT
    if _NC is None:
        _NC = _build()
    in_maps, be3_sum = _prep(inputs)
    trace = bool(os.environ.get("BASS_KERNEL_TRACE"))
    kwargs = {}
    if trace:
        import concourse.bass_utils as _bu
        _bu.upload_artifacts = lambda d: d  # no artifact bucket here
        kwargs["tmpdir"] = os.environ.get("BASS_KERNEL_TRACE_DIR") or None
    res = run_bass_kernel_spmd(_NC, in_maps, list(range(N_CORES)),
                               trace=trace, **kwargs)
    LAST_RESULT = res
    logits = np.empty((B, TGT), np.float32)
    for c in range(N_CORES):
        s = res.results[c]["out"]  # [TGT, BPC]
        logits[c * BPC:(c + 1) * BPC] = ((s + be3_sum[:, None] * T) / (E * T)).T
    return logits


# revision 13
# speedup vs baseline: 1.0043x; 1.0043x over previous
"""Trainium2 Bass kernel for the GPT2Shared multimodal ensemble MLP.

Pipeline (per token): three modality adapters (Linear+GELU) -> shared
projection -> concat -> 32-expert ensemble MLP (2304->768->32->5, relu) ->
mean over experts -> mean over time.

Sharding: pure data-parallel over the batch dim. Each of the 8 cores gets
4 batches (1024 tokens) and runs the whole pipeline for its tokens; the
final reduction over experts+time happens on-device, so each core emits a
[5, 4] partial and the host only rescales/concats.

All on-device tensors live in [feature, token] layout so every matmul uses
the natural weight layout as the stationary operand and no transposes are
needed anywhere.

Precision: the adapters, shared projection, and the dominant ensemble
first layer (2304->768 per expert, ~79% of all FLOPs) run as fp8-e4m3
DoubleRow matmuls (2 fp8 weights per PE cell -> 256-deep contraction per
pass; HW-calibrated 244 ns per K=256/N=512 matmul vs 215 ns for a bf16
K=128/N=512 one, i.e. ~1.75x FLOP throughput). Weights are pre-scaled by
2^7 and the chunk by 2^4 on the way into fp8 so values sit in e4m3's
normal range; the scales are divided back out (exact powers of two)
inside the consuming activation instruction. The narrow ensemble layers
2/3 stay bf16 - fp8 there triples the final error for <4% of the
compute. Layer 2 (768->32 per expert) packs 4 experts into one PE pass
via 32-column tile_position groups. PSUM accumulation is fp32
throughout. Measured end-to-end rel err 6.96e-3 (gate 2e-2).
"""

import os
import sys

for _p in ("/opt/trn_rl_repo", "/root/.axon_site/_ro/trn_rl_repo"):
    if os.path.isdir(_p) and _p not in sys.path:
        sys.path.append(_p)

import ml_dtypes
import numpy as np

import concourse.bass as bass
import concourse.tile as tile
from concourse import bacc, mybir
from concourse.bass_utils import run_bass_kernel_spmd

BF16 = mybir.dt.bfloat16
F8 = mybir.dt.float8e4
F32 = mybir.dt.float32
NPBF = ml_dtypes.bfloat16
NPF8 = ml_dtypes.float8_e4m3

N_CORES = 8
B, T = 32, 256
TOK = B * T // N_CORES          # 1024 tokens per core
BPC = B // N_CORES              # 4 batches per core
NT, NSZ = 2, 512                # token tiles per core
GKT = 18                        # 2304 gelu/chunk features = 18 x 128 rows
GK2 = 9                         # ... = 9 x 256 DoubleRow k-tiles
PKT = 6                         # 768 proj features = 6 x 128 rows
E, HID, TGT = 32, 32, 5
SW = 128.0                      # fp8 weight pre-scale (2^7)
SC = 16.0                       # fp8 chunk pre-scale (2^4)
# (name, in_dim, in 256-k-tiles, chunk row-tile offset) in reference
# concat order: chunk = [video, text, audio]
MODS = (("v", 768, 3, 0), ("t", 768, 3, 6), ("a", 1024, 4, 12))

_NC = None
LAST_RESULT = None


def _build():
    nc = bacc.Bacc("TRN2", target_bir_lowering=False, debug=False,
                   num_devices=N_CORES)
    DR = mybir.MatmulPerfMode.DoubleRow

    dr = {}
    for mn, kin, k2t, _ in MODS:
        dr[f"x{mn}"] = nc.dram_tensor(f"x{mn}", [128, k2t, 2, TOK], F8,
                                      kind="ExternalInput")
        dr[f"W{mn}"] = nc.dram_tensor(f"W{mn}", [128, k2t, 2, 2304], F8,
                                      kind="ExternalInput")
        dr[f"b{mn}"] = nc.dram_tensor(f"b{mn}", [128, GKT], F32,
                                      kind="ExternalInput")
    dr["Wp"] = nc.dram_tensor("Wp", [128, GK2, 2, 768], F8, kind="ExternalInput")
    dr["bp"] = nc.dram_tensor("bp", [128, PKT], F32, kind="ExternalInput")
    dr["We1"] = nc.dram_tensor("We1", [E, 128, GK2, 2, 768], F8,
                               kind="ExternalInput")
    dr["be1"] = nc.dram_tensor("be1", [128, E, PKT], F32, kind="ExternalInput")
    # host-rearranged: [p, kt, e, h] <- We2[e, kt*128+p, h]
    dr["We2"] = nc.dram_tensor("We2", [128, PKT, E, HID], BF16, kind="ExternalInput")
    # 4-expert-packed layer-2 bias: be2_pack[q*32+h, g] = be2[4g+q, h]
    dr["be2"] = nc.dram_tensor("be2", [128, 8], F32, kind="ExternalInput")
    # host-stacked: [p, kt, t] <- We3[(kt*128+p)//32, (kt*128+p)%32, t]
    dr["We3"] = nc.dram_tensor("We3", [128, 8, TGT], F32, kind="ExternalInput")
    out_d = nc.dram_tensor("out", [TGT, BPC], F32, kind="ExternalOutput")

    gelu = mybir.ActivationFunctionType.Gelu_apprx_tanh
    relu = mybir.ActivationFunctionType.Relu
    ident = mybir.ActivationFunctionType.Identity

    with tile.TileContext(nc) as tc:
        with (
            tc.tile_pool(name="const", bufs=1) as constp,
            tc.tile_pool(name="persist", bufs=1) as perp,
            tc.tile_pool(name="adw", bufs=1) as adw,
            tc.tile_pool(name="adwm", bufs=1) as adwm,
            tc.tile_pool(name="adf", bufs=2) as adf,
            tc.tile_pool(name="we1p", bufs=2) as we1p,
            tc.tile_pool(name="h1p", bufs=5) as h1p,
            tc.tile_pool(name="psA", bufs=5, space=bass.MemorySpace.PSUM) as psA,
            tc.tile_pool(name="psB", bufs=2, space=bass.MemorySpace.PSUM) as psB,
            tc.tile_pool(name="psC", bufs=1, space=bass.MemorySpace.PSUM) as psC,
        ):
            # chunk (= concat of the three projected modalities), fp8,
            # scaled by SC, in DoubleRow k-pair layout [p, k2, i, tok]
            chunk_sb = perp.tile([128, GK2, 2, TOK], F8, tag="chunk")

            # ---------------- adapters + shared projection ----------------
            # DMA issue order puts the first modality's weights/features
            # first so the PE can start ~4us into the kernel; constants and
            # Wp follow (not needed until the first gelu psum drains).
            first = True
            for mn, kin, k2t, coff in MODS:
                wm_sb = adwm.tile([128, 4, 2, 2304], F8, tag="wmod")
                f_sb = adf.tile([128, 4, 2, TOK], F8, tag="feat")
                # split per k2-tile: the first matmul only needs slice 0
                for kt in range(k2t):
                    nc.sync.dma_start(wm_sb[:, kt:kt + 1], dr[f"W{mn}"][:, kt:kt + 1])
                    nc.sync.dma_start(f_sb[:, kt:kt + 1], dr[f"x{mn}"][:, kt:kt + 1])
                bm_sb = constp.tile([128, GKT], F32, tag=f"b{mn}")
                nc.sync.dma_start(bm_sb[:], dr[f"b{mn}"][:])
                if first:
                    first = False
                    wp_sb = adw.tile([128, GK2, 2, 768], F8, tag="wp")
                    nc.sync.dma_start(wp_sb[:], dr["Wp"][:])
                    bp_sb = constp.tile([128, PKT], F32, tag="bp")
                    nc.sync.dma_start(bp_sb[:], dr["bp"][:])
                    be1_sb = constp.tile([128, E, PKT], F32, tag="be1")
                    nc.sync.dma_start(be1_sb[:], dr["be1"][:])
                    we2_sb = constp.tile([128, PKT, E, HID], BF16, tag="we2")
                    nc.sync.dma_start(we2_sb[:], dr["We2"][:])
                    be2_sb = constp.tile([128, 8], F32, tag="be2")
                    nc.sync.dma_start(be2_sb[:], dr["be2"][:])
                    we3_sb = constp.tile([128, 8, TGT], F32, tag="we3")
                    nc.sync.dma_start(we3_sb[:], dr["We3"][:])
                g_sb = adw.tile([128, GK2, 2, TOK], F8, tag="g")
                # g = gelu(x @ Wm + bm), fp8 unscaled, [feature, token]
                for n in range(NT):
                    for gf in range(GKT):
                        ps = psA.tile([128, NSZ], F32, tag="ps")
                        for kt in range(k2t):
                            nc.tensor.matmul(
                                ps[:],
                                wm_sb[:, kt, :, gf * 128:(gf + 1) * 128],
                                f_sb[:, kt, :, n * NSZ:(n + 1) * NSZ],
                                start=(kt == 0), stop=(kt == k2t - 1),
                                perf_mode=DR)
                        nc.scalar.activation(
                            g_sb[:, gf // 2, gf % 2, n * NSZ:(n + 1) * NSZ],
                            ps[:], gelu, bias=bm_sb[:, gf:gf + 1],
                            scale=1.0 / SW)
                # chunk row-tiles [coff:coff+6] = SC * (g @ Wp + bp)
                for n in range(NT):
                    for pf in range(PKT):
                        ps = psA.tile([128, NSZ], F32, tag="ps")
                        for kt in range(GK2):
                            nc.tensor.matmul(
                                ps[:],
                                wp_sb[:, kt, :, pf * 128:(pf + 1) * 128],
                                g_sb[:, kt, :, n * NSZ:(n + 1) * NSZ],
                                start=(kt == 0), stop=(kt == GK2 - 1),
                                perf_mode=DR)
                        r = coff + pf
                        # identity w/ scale+bias on the (otherwise idle) DVE
                        # so the ACT engine keeps pace with the gelu stream
                        nc.vector.tensor_scalar(
                            chunk_sb[:, r // 2, r % 2, n * NSZ:(n + 1) * NSZ],
                            ps[:], SC / SW, bp_sb[:, pf:pf + 1],
                            mybir.AluOpType.mult, mybir.AluOpType.add)

            # ---------------- ensemble ----------------
            h2_sb = perp.tile([128, 8, TOK], BF16, tag="h2")
            hs_sb = perp.tile([128, 8, BPC], F32, tag="hs")
            if True:
                h1_grp = []
                for e in range(E):
                    w1_sb = we1p.tile([128, GK2, 2, 768], F8, tag="w1")
                    nc.sync.dma_start(w1_sb[:], dr["We1"][e])
                    h1_sb = h1p.tile([128, PKT, TOK], BF16, tag="h1")
                    for n in range(NT):
                        for pf in range(PKT):
                            ps = psA.tile([128, NSZ], F32, tag="ps")
                            for kt in range(GK2):
                                nc.tensor.matmul(
                                    ps[:],
                                    w1_sb[:, kt, :, pf * 128:(pf + 1) * 128],
                                    chunk_sb[:, kt, :, n * NSZ:(n + 1) * NSZ],
                                    start=(kt == 0), stop=(kt == GK2 - 1),
                                    perf_mode=DR)
                            nc.scalar.activation(
                                h1_sb[:, pf, n * NSZ:(n + 1) * NSZ], ps[:],
                                relu, bias=be1_sb[:, e, pf:pf + 1],
                                scale=1.0 / (SC * SW))
                    h1_grp.append(h1_sb)
                    if e % 4 != 3:
                        continue
                    # layer 2 for experts 4g..4g+3 packed into one PE pass:
                    # each expert's [128,32] We2 sits in its own 32-column
                    # group of the array (tile_position from the psum
                    # quarter), so the 4 matmuls per k-tile run concurrently
                    g = e // 4
                    for n in range(NT):
                        ps2 = psB.tile([128, NSZ], F32, tag="ps2")
                        for kt in range(PKT):
                            for q in range(4):
                                nc.tensor.matmul(
                                    ps2[q * 32:(q + 1) * 32, :],
                                    we2_sb[:, kt, 4 * g + q, :],
                                    h1_grp[q][:, kt, n * NSZ:(n + 1) * NSZ],
                                    start=(kt == 0), stop=(kt == PKT - 1),
                                    tile_position=(0, q * 32))
                        nc.scalar.activation(
                            h2_sb[:, g, n * NSZ:(n + 1) * NSZ],
                            ps2[:], relu, bias=be2_sb[:, g:g + 1])
                    # time-reduce this group's h2 now (idle DVE, overlaps
                    # the next group's layer-1 matmuls); layer 3 is linear
                    # so sum-over-time commutes with it
                    nc.vector.reduce_sum(
                        hs_sb[:, g],
                        h2_sb[:, g].rearrange("p (b t) -> p b t", t=T),
                        axis=mybir.AxisListType.X)
                    h1_grp = []

                # ensemble head on the tiny time sums: out = sum_T h2 @ We3
                s_sb = constp.tile([TGT, BPC], F32, tag="s")
                ps3 = psC.tile([TGT, BPC], F32, tag="ps3")
                for kt in range(8):
                    nc.tensor.matmul(
                        ps3[:],
                        we3_sb[:, kt, :],
                        hs_sb[:, kt, :],
                        start=(kt == 0), stop=(kt == 7))
                nc.any.tensor_copy(s_sb[:], ps3[:])
                nc.sync.dma_start(out_d[:], s_sb[:])

    nc.compile()
    return nc


def _pair128(a):
    """[K, M] -> [128, K//256, 2, M]: DoubleRow k-pair layout, partition
    dim first."""
    K, M = a.shape
    return np.ascontiguousarray(
        a.reshape(K // 256, 2, 128, M).transpose(2, 0, 1, 3))


def _prep(inputs):
    """Host-side: cast/scale to fp8 or bf16, transpose feats to
    [feature, token], build per-core input maps."""
    f32 = np.float32

    def bf(x):
        return np.asarray(x, f32).astype(NPBF)

    def f8(x, s=1.0):
        return (np.asarray(x, f32) * f32(s)).astype(NPF8)

    feats = {
        "v": np.asarray(inputs["video_feat"], f32).reshape(B * T, 768),
        "t": np.asarray(inputs["text_feat"], f32).reshape(B * T, 768),
        "a": np.asarray(inputs["audio_feat"], f32).reshape(B * T, 1024),
    }
    # [128, k2t, 2, N] fp8, unscaled (randn fits e4m3 range directly)
    featsT = {k: _pair128(f8(v.T)) for k, v in feats.items()}

    wkeys = {"v": "Wv", "t": "Wt", "a": "Wa"}
    bkeys = {"v": "bv", "t": "bt", "a": "ba"}
    shared = {}
    for mn, kin, _, _ in MODS:
        shared[f"W{mn}"] = _pair128(f8(inputs[wkeys[mn]], SW))
        shared[f"b{mn}"] = np.ascontiguousarray(
            np.asarray(inputs[bkeys[mn]], f32).reshape(GKT, 128).T)
    shared["Wp"] = _pair128(f8(inputs["Wp"], SW))
    shared["bp"] = np.ascontiguousarray(
        np.asarray(inputs["bp"], f32).reshape(PKT, 128).T * f32(SC))
    w1 = f8(inputs["We1"], SW)  # [E, 2304, 768]
    shared["We1"] = np.ascontiguousarray(
        w1.reshape(E, GK2, 2, 128, 768).transpose(0, 3, 1, 2, 4))
    shared["be1"] = np.ascontiguousarray(
        np.asarray(inputs["be1"], f32).reshape(E, PKT, 128).transpose(2, 0, 1))
    shared["We2"] = np.ascontiguousarray(
        bf(inputs["We2"]).reshape(E, PKT, 128, HID).transpose(2, 1, 0, 3))
    shared["be2"] = np.ascontiguousarray(
        np.asarray(inputs["be2"], f32).reshape(8, 4, HID)
        .transpose(1, 2, 0).reshape(128, 8))
    shared["We3"] = np.ascontiguousarray(
        np.asarray(inputs["We3"], f32).reshape(8, 128, TGT).transpose(1, 0, 2))

    in_maps = []
    for c in range(N_CORES):
        m = dict(shared)
        sl = slice(c * TOK, (c + 1) * TOK)
        for mn, _, _, _ in MODS:
            m[f"x{mn}"] = np.ascontiguousarray(featsT[mn][:, :, :, sl])
        in_maps.append(m)
    be3_sum = np.asarray(inputs["be3"], f32).sum(axis=0)
    return in_maps, be3_sum


def kernel(**inputs):
    global _NC, LAST_RESULT
    if _NC is None:
        _NC = _build()
    in_maps, be3_sum = _prep(inputs)
    trace = bool(os.environ.get("BASS_KERNEL_TRACE"))
    kwargs = {}
    if trace:
        import concourse.bass_utils as _bu
        _bu.upload_artifacts = lambda d: d  # no artifact bucket here
        kwargs["tmpdir"] = os.environ.get("BASS_KERNEL_TRACE_DIR") or None
    res = run_bass_kernel_spmd(_NC, in_maps, list(range(N_CORES)),
                               trace=trace, **kwargs)
    LAST_RESULT = res
    logits = np.empty((B, TGT), np.float32)
    for c in range(N_CORES):
        s = res.results[c]["out"]  # [TGT, BPC]
        logits[c * BPC:(c + 1) * BPC] = ((s + be3_sum[:, None] * T) / (E * T)).T
    return logits
